# revision 9
# baseline (speedup 1.0000x reference)
"""MinGRU cell kernel for Trainium2 (8 NeuronCores, data-parallel over batch).

Computes, for x:[B,T,D], motion_mag:[B,T]:
    tau = 1 + softplus(alpha) * sigmoid(mw*mm + mb)        (per b,t)
    z   = sigmoid((x @ Wz^T + bz) / tau)                   (B,T,H)
    ht  = x @ Wh^T + bh                                    (B,T,H)
    h_t = (1-z_t)*h_{t-1} + z_t*ht_t   (scan over t, h_0=0)

Strategy:
  - Shard B=32 across 8 cores (4 per core). Weights replicated.
  - On-chip layout: h on partitions, t on the free dim, so the recurrence is
    a single HW tensor_tensor_scan over the full T=2048 per [128h] tile.
  - Projections: lhsT = W^T chunks (stationary), rhs = x^T chunks (moving),
    float32r (full PE rate, fp32 PSUM accumulation).
  - tau: 1/tau computed on host, broadcast across partitions once per sample
    (gpsimd partition_broadcast); folded in via one fused scalar_tensor_tensor
    per PSUM tile: u = (zpre + bz) * invtau.
  - a = sigmoid(-u) = 1 - z and z = sigmoid(u) on ACT over [128, 2048] tiles.
  - b = (hpre + bh) * z fused on DVE; h = tensor_tensor_scan(a, b, 0).
  - Host pre-transposes x to [d, b*t] per core and un-transposes the output.
"""

import sys

import numpy as np

if "/opt/trn_rl_repo" not in sys.path:
    sys.path.insert(0, "/opt/trn_rl_repo")

B, T, D, H = 32, 2048, 512, 512
NCORES = 8
BL = B // NCORES            # batch per core = 4
TBLK = 512                  # t-columns per psum block
NTB = T // TBLK             # 4 t-blocks per sample
DC = D // 128               # 4 contraction chunks
HC = H // 128               # 4 h partition chunks
BT = BL * T                 # 8192 columns per core

_CACHE = {}


def _build_nc():
    import concourse.bass as bass
    import concourse.bacc as bacc
    import concourse.mybir as mybir
    import concourse.tile as tile
    from contextlib import ExitStack

    f32 = mybir.dt.float32
    f32r = mybir.dt.float32r
    AF = mybir.ActivationFunctionType
    OP = mybir.AluOpType

    nc = bacc.Bacc("TRN2", target_bir_lowering=False, debug=False)

    xt_ext = nc.declare_dram_parameter("xt", [DC, 128, BT], f32r, isOutput=False)
    wzt_ext = nc.declare_dram_parameter("wzt", [DC, 128, H], f32r, isOutput=False)
    wht_ext = nc.declare_dram_parameter("wht", [DC, 128, H], f32r, isOutput=False)
    bz_ext = nc.declare_dram_parameter("bz", [HC, 128, 1], f32, isOutput=False)
    bh_ext = nc.declare_dram_parameter("bh", [HC, 128, 1], f32, isOutput=False)
    itau_ext = nc.declare_dram_parameter("invtau", [BL, 1, T], f32, isOutput=False)
    out_ext = nc.declare_dram_parameter("out", [BL, HC, 128, T], f32, isOutput=True)

    with tile.TileContext(nc) as tc, ExitStack() as ctx:
        singles = ctx.enter_context(tc.tile_pool(name="singles", bufs=1))
        x_pool = ctx.enter_context(tc.tile_pool(name="x", bufs=2))
        j_pool = ctx.enter_context(tc.tile_pool(name="j", bufs=2))
        psum = ctx.enter_context(tc.tile_pool(name="psum", bufs=4, space="PSUM"))
        u_pool = ctx.enter_context(tc.tile_pool(name="u", bufs=4))
        za_pool = ctx.enter_context(tc.tile_pool(name="za", bufs=5))
        b_pool = ctx.enter_context(tc.tile_pool(name="bb", bufs=5))
        h_pool = ctx.enter_context(tc.tile_pool(name="h", bufs=2))

        # Stationary weights and bias columns.
        wz_sb, wh_sb = [], []
        for dc in range(DC):
            wz = singles.tile([128, H], f32r, tag=f"wz{dc}")
            nc.sync.dma_start(out=wz[:], in_=wzt_ext[dc])
            wz_sb.append(wz)
            wh = singles.tile([128, H], f32r, tag=f"wh{dc}")
            nc.sync.dma_start(out=wh[:], in_=wht_ext[dc])
            wh_sb.append(wh)
        bz_col, bh_col = [], []
        for hc in range(HC):
            bzc = singles.tile([128, 1], f32, tag=f"bz{hc}")
            nc.sync.dma_start(out=bzc[:], in_=bz_ext[hc])
            bz_col.append(bzc)
            bhc = singles.tile([128, 1], f32, tag=f"bh{hc}")
            nc.sync.dma_start(out=bhc[:], in_=bh_ext[hc])
            bh_col.append(bhc)

        for b in range(BL):
            # 1/tau row for this sample, broadcast to all 128 partitions.
            iv_row = j_pool.tile([1, T], f32, tag="ivrow")
            nc.sync.dma_start(out=iv_row[:], in_=itau_ext[b])
            jt = j_pool.tile([128, T], f32, tag="J")
            nc.gpsimd.partition_broadcast(jt[:], iv_row[:])

            # full-T operand rows for the scan, filled block by block
            a_big = [za_pool.tile([128, T], f32, tag="a", name=f"abig{hc}") for hc in range(HC)]
            b_big = [b_pool.tile([128, T], f32, tag="b", name=f"bbig{hc}") for hc in range(HC)]

            for tb in range(NTB):
                bt0 = b * T + tb * TBLK
                ts = slice(tb * TBLK, (tb + 1) * TBLK)
                xs = []
                for dc in range(DC):
                    xt = x_pool.tile([128, TBLK], f32r, tag=f"x{dc}")
                    nc.sync.dma_start(out=xt[:], in_=xt_ext[dc, :, bt0:bt0 + TBLK])
                    xs.append(xt)

                for hc in range(HC):
                    zq = psum.tile([128, TBLK], f32, tag="zq")
                    for dc in range(DC):
                        nc.tensor.matmul(
                            zq[:],
                            lhsT=wz_sb[dc][:, hc * 128:(hc + 1) * 128],
                            rhs=xs[dc][:],
                            start=(dc == 0),
                            stop=(dc == DC - 1),
                        )
                    hq = psum.tile([128, TBLK], f32, tag="hq")
                    for dc in range(DC):
                        nc.tensor.matmul(
                            hq[:],
                            lhsT=wh_sb[dc][:, hc * 128:(hc + 1) * 128],
                            rhs=xs[dc][:],
                            start=(dc == 0),
                            stop=(dc == DC - 1),
                        )
                    # u = (zpre + bz) * invtau
                    u = u_pool.tile([128, TBLK], f32, tag="u")
                    nc.vector.scalar_tensor_tensor(
                        u[:], zq[:], bz_col[hc][:], jt[:, ts],
                        op0=OP.add, op1=OP.mult,
                    )
                    z = u_pool.tile([128, TBLK], f32, tag="z")
                    nc.scalar.activation(z[:], u[:], AF.Sigmoid)
                    nc.scalar.activation(
                        a_big[hc][:, ts], u[:], AF.Sigmoid, scale=-1.0
                    )
                    # b = (hpre + bh) * z
                    nc.vector.scalar_tensor_tensor(
                        b_big[hc][:, ts], hq[:], bh_col[hc][:], z[:],
                        op0=OP.add, op1=OP.mult,
                    )

            for hc in range(HC):
                h = h_pool.tile([128, T], f32, tag="h")
                nc.vector.tensor_tensor_scan(
                    h[:], a_big[hc][:], b_big[hc][:], 0.0, op0=OP.mult, op1=OP.add
                )
                nc.sync.dma_start(out=out_ext[b, hc], in_=h[:])

    nc.compile()
    return nc


def _prep_inputs(x, motion_mag, Wz, bz, Wh, bh, motion_weight, motion_bias, alpha):
    x = np.ascontiguousarray(np.asarray(x, dtype=np.float32))
    mm = np.asarray(motion_mag, dtype=np.float32)
    Wz = np.asarray(Wz, dtype=np.float32)
    Wh = np.asarray(Wh, dtype=np.float32)
    bz = np.asarray(bz, dtype=np.float32).reshape(HC, 128, 1)
    bh = np.asarray(bh, dtype=np.float32).reshape(HC, 128, 1)
    mw = float(np.asarray(motion_weight))
    mb = float(np.asarray(motion_bias))
    al = float(np.asarray(alpha))

    a_sp = float(np.log1p(np.exp(al)))  # softplus(alpha)
    sig = 1.0 / (1.0 + np.exp(-(mw * mm + mb)))
    invtau = (1.0 / (1.0 + a_sp * sig)).astype(np.float32)

    wzt = np.ascontiguousarray(Wz.T).reshape(DC, 128, H)
    wht = np.ascontiguousarray(Wh.T).reshape(DC, 128, H)

    in_maps = []
    for c in range(NCORES):
        xl = x[c * BL:(c + 1) * BL].reshape(BL * T, D)
        xt = np.ascontiguousarray(xl.T).reshape(DC, 128, BT)
        in_maps.append({
            "xt": xt,
            "wzt": wzt,
            "wht": wht,
            "bz": bz,
            "bh": bh,
            "invtau": np.ascontiguousarray(
                invtau[c * BL:(c + 1) * BL]).reshape(BL, 1, T),
        })
    return in_maps


def _assemble(results):
    outs = []
    for c in range(NCORES):
        o = results[c]["out"]  # [BL, HC, 128, T]
        o = np.transpose(o, (0, 3, 1, 2)).reshape(BL, T, H)
        outs.append(o)
    return np.ascontiguousarray(np.concatenate(outs, axis=0))


def _run(inputs, trace=False):
    from concourse.bass_utils import run_bass_kernel_spmd

    if "nc" not in _CACHE:
        _CACHE["nc"] = _build_nc()
    nc = _CACHE["nc"]
    in_maps = _prep_inputs(**inputs)
    res = run_bass_kernel_spmd(nc, in_maps, list(range(NCORES)), trace=trace)
    return _assemble(res.results), res


def kernel(**inputs):
    out, _ = _run(inputs, trace=False)
    return out


# revision 12
# speedup vs baseline: 1.2634x; 1.2634x over previous
"""MinGRU cell kernel for Trainium2 (8 NeuronCores, data-parallel over batch).

Computes, for x:[B,T,D], motion_mag:[B,T]:
    tau = 1 + softplus(alpha) * sigmoid(mw*mm + mb)        (per b,t)
    z   = sigmoid((x @ Wz^T + bz) / tau)                   (B,T,H)
    ht  = x @ Wh^T + bh                                    (B,T,H)
    h_t = (1-z_t)*h_{t-1} + z_t*ht_t   (scan over t, h_0=0)

Strategy:
  - Shard B=32 across 8 cores (4 per core). Weights replicated.
  - On-chip layout: h on partitions, t on the free dim, so the recurrence is
    one HW tensor_tensor_scan over the full T=2048 per [128h] row-block.
  - Projections in bf16 (x, W cast on host) with exact fp32 PSUM accumulation.
  - tau: 1/tau computed on host, DMA-broadcast across partitions per sample;
    u = (zpre + bz) * invtau fused on DVE, written to PSUM so both sigmoids
    read PSUM (fast ScalarE path): z = sigmoid(u), a = sigmoid(-u) = 1-z.
  - Candidate: ht = hq + bh on ScalarE (PSUM read); b = ht * z on GPSIMD
    (keeps the DVE free for the scans).
  - h = tensor_tensor_scan(a, b, 0) on DVE; bf16 scan operands, fp32 output.
  - Host pre-transposes x to [d, b*t] per core and un-transposes the output.
"""

import sys

import numpy as np

if "/opt/trn_rl_repo" not in sys.path:
    sys.path.insert(0, "/opt/trn_rl_repo")

B, T, D, H = 32, 2048, 512, 512
NCORES = 8
BL = B // NCORES            # batch per core = 4
TBLK = 512                  # t-columns per psum block
NTB = T // TBLK             # 4 t-blocks per sample
DC = D // 128               # 4 contraction chunks
HC = H // 128               # 4 h partition chunks
BT = BL * T                 # 8192 columns per core

_CACHE = {}


def _build_nc(bh0=None):
    bh_uniform = bh0 is not None
    import concourse.bass as bass
    import concourse.bacc as bacc
    import concourse.mybir as mybir
    import concourse.tile as tile
    from contextlib import ExitStack

    f32 = mybir.dt.float32
    bf16 = mybir.dt.bfloat16
    AF = mybir.ActivationFunctionType
    OP = mybir.AluOpType

    nc = bacc.Bacc("TRN2", target_bir_lowering=False, debug=False)

    xt_ext = nc.declare_dram_parameter("xt", [DC, 128, BT], bf16, isOutput=False)
    wzt_ext = nc.declare_dram_parameter("wzt", [DC, 128, H], bf16, isOutput=False)
    wht_ext = nc.declare_dram_parameter("wht", [DC, 128, H], bf16, isOutput=False)
    bz_ext = nc.declare_dram_parameter("bz", [HC, 128, 1], f32, isOutput=False)
    bh_ext = nc.declare_dram_parameter("bh", [HC, 128, 1], f32, isOutput=False)
    itau_ext = nc.declare_dram_parameter("invtau", [BL, 1, T], f32, isOutput=False)
    out_ext = nc.declare_dram_parameter("out", [BL, HC, 128, T], f32, isOutput=True)

    with tile.TileContext(nc) as tc, ExitStack() as ctx:
        singles = ctx.enter_context(tc.tile_pool(name="singles", bufs=1))
        x_pool = ctx.enter_context(tc.tile_pool(name="x", bufs=3))
        j_pool = ctx.enter_context(tc.tile_pool(name="j", bufs=2))
        psum = ctx.enter_context(tc.tile_pool(name="psum", bufs=3, space="PSUM"))
        upsum = ctx.enter_context(tc.tile_pool(name="upsum", bufs=2, space="PSUM"))
        zh_pool = ctx.enter_context(tc.tile_pool(name="zh", bufs=4))
        za_pool = ctx.enter_context(tc.tile_pool(name="za", bufs=5))
        b_pool = ctx.enter_context(tc.tile_pool(name="bb", bufs=5))
        h_pool = ctx.enter_context(tc.tile_pool(name="h", bufs=2))

        # Stationary weights and bias columns.
        wz_sb, wh_sb = [], []
        for dc in range(DC):
            wz = singles.tile([128, H], bf16, tag=f"wz{dc}")
            nc.sync.dma_start(out=wz[:], in_=wzt_ext[dc])
            wz_sb.append(wz)
            wh = singles.tile([128, H], bf16, tag=f"wh{dc}")
            nc.sync.dma_start(out=wh[:], in_=wht_ext[dc])
            wh_sb.append(wh)
        bz_col, bh_col = [], []
        for hc in range(HC):
            bzc = singles.tile([128, 1], f32, tag=f"bz{hc}")
            nc.sync.dma_start(out=bzc[:], in_=bz_ext[hc])
            bz_col.append(bzc)
            bhc = singles.tile([128, 1], f32, tag=f"bh{hc}")
            nc.sync.dma_start(out=bhc[:], in_=bh_ext[hc])
            bh_col.append(bhc)

        for b in range(BL):
            # 1/tau row broadcast across partitions straight from DRAM.
            jt = j_pool.tile([128, T], f32, tag="J")
            iv = itau_ext[b, 0]
            iv_b = bass.AP(
                tensor=iv.tensor, offset=iv.offset, ap=[[0, 128]] + list(iv.ap)
            )
            nc.gpsimd.dma_start(out=jt[:], in_=iv_b)

            # full-T scan operands, filled block by block
            a_big = [za_pool.tile([128, T], bf16, tag="a", name=f"abig{hc}")
                     for hc in range(HC)]
            b_big = [b_pool.tile([128, T], bf16, tag="b", name=f"bbig{hc}")
                     for hc in range(HC)]

            for tb in range(NTB):
                bt0 = b * T + tb * TBLK
                ts = slice(tb * TBLK, (tb + 1) * TBLK)
                xs = []
                for dc in range(DC):
                    xt = x_pool.tile([128, TBLK], bf16, tag=f"x{dc}")
                    nc.sync.dma_start(out=xt[:], in_=xt_ext[dc, :, bt0:bt0 + TBLK])
                    xs.append(xt)

                for hc in range(HC):
                    zq = psum.tile([128, TBLK], f32, tag="zq")
                    for dc in range(DC):
                        nc.tensor.matmul(
                            zq[:],
                            lhsT=wz_sb[dc][:, hc * 128:(hc + 1) * 128],
                            rhs=xs[dc][:],
                            start=(dc == 0),
                            stop=(dc == DC - 1),
                        )
                    hq = psum.tile([128, TBLK], f32, tag="hq")
                    for dc in range(DC):
                        nc.tensor.matmul(
                            hq[:],
                            lhsT=wh_sb[dc][:, hc * 128:(hc + 1) * 128],
                            rhs=xs[dc][:],
                            start=(dc == 0),
                            stop=(dc == DC - 1),
                        )
                    # u = (zpre + bz) * invtau  (to PSUM: ScalarE reads PSUM fast)
                    u = upsum.tile([128, TBLK], f32, tag="u")
                    nc.vector.scalar_tensor_tensor(
                        u[:], zq[:], bz_col[hc][:], jt[:, ts],
                        op0=OP.add, op1=OP.mult,
                    )
                    z = zh_pool.tile([128, TBLK], bf16, tag="z")
                    nc.scalar.activation(z[:], u[:], AF.Sigmoid)
                    nc.scalar.activation(
                        a_big[hc][:, ts], u[:], AF.Sigmoid, scale=-1.0
                    )
                    # ht = hq + bh (ScalarE, PSUM read), b = ht * z (GPSIMD)
                    if bh_uniform:
                        ht = zh_pool.tile([128, TBLK], bf16, tag="ht")
                        nc.scalar.activation(
                            ht[:], hq[:], AF.Copy, bias=bh0
                        )
                        nc.gpsimd.tensor_tensor(
                            b_big[hc][:, ts], ht[:], z[:], op=OP.mult
                        )
                    else:
                        nc.vector.scalar_tensor_tensor(
                            b_big[hc][:, ts], hq[:], bh_col[hc][:], z[:],
                            op0=OP.add, op1=OP.mult,
                        )

            for hc in range(HC):
                h = h_pool.tile([128, T], f32, tag="h")
                nc.vector.tensor_tensor_scan(
                    h[:], a_big[hc][:], b_big[hc][:], 0.0, op0=OP.mult, op1=OP.add
                )
                nc.sync.dma_start(out=out_ext[b, hc], in_=h[:])

    nc.compile()
    return nc


def _prep_inputs(x, motion_mag, Wz, bz, Wh, bh, motion_weight, motion_bias, alpha):
    import ml_dtypes

    bfloat16 = ml_dtypes.bfloat16
    x = np.ascontiguousarray(np.asarray(x, dtype=np.float32))
    mm = np.asarray(motion_mag, dtype=np.float32)
    Wz = np.asarray(Wz, dtype=np.float32)
    Wh = np.asarray(Wh, dtype=np.float32)
    bz = np.asarray(bz, dtype=np.float32).reshape(HC, 128, 1)
    bh = np.asarray(bh, dtype=np.float32).reshape(HC, 128, 1)
    mw = float(np.asarray(motion_weight))
    mb = float(np.asarray(motion_bias))
    al = float(np.asarray(alpha))

    a_sp = float(np.log1p(np.exp(al)))  # softplus(alpha)
    sig = 1.0 / (1.0 + np.exp(-(mw * mm + mb)))
    invtau = (1.0 / (1.0 + a_sp * sig)).astype(np.float32)

    wzt = np.ascontiguousarray(Wz.T).reshape(DC, 128, H).astype(bfloat16)
    wht = np.ascontiguousarray(Wh.T).reshape(DC, 128, H).astype(bfloat16)

    in_maps = []
    for c in range(NCORES):
        xl = x[c * BL:(c + 1) * BL].reshape(BL * T, D)
        xt = np.ascontiguousarray(xl.T).reshape(DC, 128, BT).astype(bfloat16)
        in_maps.append({
            "xt": xt,
            "wzt": wzt,
            "wht": wht,
            "bz": bz,
            "bh": bh,
            "invtau": np.ascontiguousarray(
                invtau[c * BL:(c + 1) * BL]).reshape(BL, 1, T),
        })
    return in_maps


def _assemble(results):
    outs = []
    for c in range(NCORES):
        o = results[c]["out"]  # [BL, HC, 128, T]
        o = np.transpose(o, (0, 3, 1, 2)).reshape(BL, T, H)
        outs.append(o)
    return np.ascontiguousarray(np.concatenate(outs, axis=0))


def _run(inputs, trace=False):
    from concourse.bass_utils import run_bass_kernel_spmd

    bh = np.asarray(inputs["bh"], dtype=np.float32).reshape(-1)
    bh0 = float(bh[0]) if np.all(bh == bh[0]) else None
    key = ("nc", bh0)
    if key not in _CACHE:
        _CACHE[key] = _build_nc(bh0)
    nc = _CACHE[key]
    in_maps = _prep_inputs(**inputs)
    res = run_bass_kernel_spmd(nc, in_maps, list(range(NCORES)), trace=trace)
    return _assemble(res.results), res


def kernel(**inputs):
    out, _ = _run(inputs, trace=False)
    return out
